# revision 10
# baseline (speedup 1.0000x reference)
"""Trainium2 Bass kernel: FiLM modulation + batched block-diagonal scatter.

Reference computation (per batch row):
    gb    = x_cond @ W + b                       # [172]
    gamma = gb[:86]; beta = gb[86:]
    out3d = (1 + gamma) * x_to_film + beta       # [256, 86]
    result[t, c] = block-diagonal placement: rows 0:86 -> cols 0:86,
                   rows 86:172 -> cols 86:172, rows 172:256 -> cols 172:256
                   (last block truncated to 84 cols); everything else zero.

Strategy: pure data parallel over the batch dim (1024 -> 8 cores x 128 rows).
Per core, batch rows live on the 128 SBUF partitions.

Performance structure (v2, bf16 datapath):
  - The film ops are DVE tensor_tensor; with every operand bf16 and packed
    along the innermost dim they run in the DVE 2x_1p perf mode (0.5
    cycles/elem instead of 1), halving the ~46us fp32 DVE floor to ~23us.
    x_to_film is loaded f32 from HBM and downcast to bf16 on the otherwise
    idle ACT engine (~19us, fully overlapped with DVE).
  - gb = x_cond @ W + b runs on PE in bf16 (1 cycle/row vs 4 for fp32).
    x_cond is pre-transposed on the host (pure layout change) so no PE
    transpose / PSUM round-trip sits on the critical path; gamma's PSUM
    accumulation group is separate from beta's so the first film multiply
    can start as soon as gamma lands.
  - Output blocks are written unpadded (86/84 cols = 172/168B descriptors).
    Sub-512B descriptors pay the documented 2x read-modify-write penalty,
    but at bf16 that equals the f32 padded-row cost with half the SBUF
    footprint and no margin-zeroing ops.
  - Three DMA queues (SP + ACT HWDGE, Pool SWDGE) transfer in parallel;
    same-queue transfers serialize, so loads/stores are spread across all
    three with the chunk splits and ring strings below (tuned via random
    search over the CoreSim cost model).
"""

import numpy as np

import concourse.bacc as bacc
import concourse.mybir as mybir
from concourse.bass_utils import run_bass_kernel_spmd

try:  # ml_dtypes provides the numpy bfloat16
    from ml_dtypes import bfloat16 as np_bf16
except ImportError:  # pragma: no cover
    import jax.numpy as jnp

    np_bf16 = jnp.bfloat16
from concourse.tile import TileContext

B, T, D_COND, D_OUT = 1024, 256, 768, 86
N_CORES = 8
BL = B // N_CORES  # 128 batch rows per core = SBUF partition count
KT = D_COND // 128  # 6 contraction tiles

# block structure of the output: (t_start, t_end, col_start, width)
BLOCKS = [(0, 86, 0, 86), (86, 172, 86, 86), (172, 256, 172, 84)]


def make_chunks(splits):
    """splits[b] = list of row counts for block b -> (t0, nt, c0, wd)."""
    chunks = []
    for (tb, te, c0, wd), ns in zip(BLOCKS, splits):
        assert sum(ns) == te - tb
        t = tb
        for n in ns:
            chunks.append((t, n, c0, wd))
            t += n
    return chunks


DEFAULT_CFG = {
    "splits": [[16, 35, 35], [43, 43], [42, 28, 14]],
    # per-chunk engine strings: S=sync(SP) A=scalar(ACT) P=gpsimd(Pool)
    "in_ring": "APPPPPPP",
    "out_ring": "ASASASAS",
    "in_group": None,
    "wx_ring": "SA",
    "b_ring": "S",
    "lookahead": 3,
}


def build_core_module(finalize=True, cfg=DEFAULT_CFG):
    nc = bacc.Bacc(
        "TRN2", target_bir_lowering=False, debug=False, enable_asserts=False
    )
    f32 = mybir.dt.float32
    bf16 = mybir.dt.bfloat16
    mult = mybir.AluOpType.mult
    add = mybir.AluOpType.add
    chunks = make_chunks(cfg["splits"])
    # Packed gb operands, one load: wx[:, k, 0:128] = x_cond^T k-tile
    # (xct[b_part, k, b] = x_cond[b, k*128 + b_part]), wx[:, k, 128:300] =
    # W k-tile (host layout prep, pure transpose/pack).
    wx = nc.dram_tensor(
        "wx", [128, KT, 128 + 2 * D_OUT], bf16, kind="ExternalInput"
    )
    xf = nc.dram_tensor("x_to_film", [BL, T, D_OUT], bf16, kind="ExternalInput")
    # b packed with a row of ones (cols 172:300) used as the K=1 lhsT
    # for the bias rank-1 matmul - saves the on-device memset.
    bv = nc.dram_tensor("b", [1, 2 * D_OUT + 128], bf16, kind="ExternalInput")
    out = nc.dram_tensor("out", [BL, T, T], bf16, kind="ExternalOutput")

    engs = {"S": nc.sync, "A": nc.scalar, "P": nc.gpsimd, "V": nc.vector}

    with TileContext(nc) as tc:
        with (
            tc.tile_pool(name="persist", bufs=1) as persist,
            tc.tile_pool(name="gbps", bufs=1, space="PSUM") as gbps,
            tc.tile_pool(name="work", bufs=3) as work,
        ):
            # --- gb = x_cond @ W + b (PE, bf16 operands, f32 PSUM accum) ---
            # gamma and beta accumulate in separate PSUM groups so gamma (the
            # first film operand needed) posts without waiting for beta.
            g1_bf = persist.tile([128, D_OUT], bf16, tag="g1")
            be_bf = persist.tile([128, D_OUT], bf16, tag="be")
            with tc.tile_pool(name="setup", bufs=1) as setup:
                wx_sb = setup.tile([128, KT, 128 + 2 * D_OUT], bf16)
                wxr = cfg["wx_ring"]
                if len(wxr) == 1:
                    engs[wxr].dma_start(out=wx_sb, in_=wx[:, :, :])
                else:
                    h = KT // 2
                    engs[wxr[0]].dma_start(
                        out=wx_sb[:, 0:h, :], in_=wx[:, 0:h, :]
                    )
                    engs[wxr[1]].dma_start(
                        out=wx_sb[:, h:KT, :], in_=wx[:, h:KT, :]
                    )
                xct_sb = wx_sb[:, :, 0:128]
                w_sb = wx_sb[:, :, 128:]
                b_sb = setup.tile([1, 2 * D_OUT + 128], bf16)
                engs[cfg["b_ring"]].dma_start(out=b_sb, in_=bv[:, :])
                ones = b_sb[:, 2 * D_OUT :]

                g_ps = gbps.tile([128, D_OUT], f32, tag="g_ps")
                b_ps = gbps.tile([128, D_OUT], f32, tag="b_ps")
                for k in range(KT):
                    nc.tensor.matmul(
                        g_ps,
                        xct_sb[:, k, :],
                        w_sb[:, k, 0:D_OUT],
                        start=(k == 0),
                        stop=False,
                    )
                nc.tensor.matmul(
                    g_ps, ones, b_sb[:, 0:D_OUT], start=False, stop=True
                )
                # gb[:, :86] -> 1+gamma (bf16), for the film multiply
                nc.scalar.add(g1_bf, g_ps, 1.0)
                for k in range(KT):
                    nc.tensor.matmul(
                        b_ps,
                        xct_sb[:, k, :],
                        w_sb[:, k, D_OUT:],
                        start=(k == 0),
                        stop=False,
                    )
                nc.tensor.matmul(
                    b_ps, ones, b_sb[:, D_OUT : 2 * D_OUT], start=False, stop=True
                )
                nc.scalar.copy(be_bf, b_ps)

            # --- FiLM + block writes ---
            # Per chunk: f32 load -> ACT downcast to bf16 -> two DVE
            # tensor_tensor passes in 2x mode -> unpadded block write.
            obufs = []
            for i, (t0, nt, c0, wd) in enumerate(chunks):
                ob = persist.tile([128, nt, wd], bf16, tag=f"obuf{i}")
                obufs.append(ob)
            # Input loads are decoupled from film chunks: x_to_film is
            # contiguous in t, so one load can span several film chunks
            # (and block boundaries), amortizing the ~1us per-DMA ring
            # overhead. cfg["in_group"][g] = number of consecutive film
            # chunks covered by load g.
            in_group = cfg.get("in_group") or [1] * len(chunks)
            assert sum(in_group) == len(chunks)
            groups = []  # (first_chunk, n_chunks, t0, nt_total)
            ci = 0
            for g, cnt in enumerate(in_group):
                t0 = chunks[ci][0]
                ntt = sum(c[1] for c in chunks[ci : ci + cnt])
                groups.append((ci, cnt, t0, ntt))
                ci += cnt
            group_of = {}
            for g, (c0i, cnt, t0, ntt) in enumerate(groups):
                for i in range(c0i, c0i + cnt):
                    group_of[i] = g
            xbufs = [
                persist.tile(
                    [128, ntt, D_OUT], bf16, tag=f"xb{g}", name=f"xb{g}"
                )
                for g, (c0i, cnt, t0, ntt) in enumerate(groups)
            ]
            # Emission order sets per-ring DMA queue order; a queued DMA
            # whose data isn't ready blocks its ring, so loads are emitted
            # `lookahead` film-chunks ahead of the film ops consuming them.
            look = cfg.get("lookahead", 3)
            emitted = set()

            def emit_in_for(i):
                g = group_of[min(i, len(chunks) - 1)]
                if g in emitted:
                    return
                emitted.add(g)
                _, _, gt0, gnt = groups[g]
                engs[cfg["in_ring"][g]].dma_start(
                    out=xbufs[g], in_=xf[:, gt0 : gt0 + gnt, :]
                )

            for j in range(min(look, len(chunks))):
                emit_in_for(j)
            for i, (t0, nt, c0, wd) in enumerate(chunks):
                g = group_of[i]
                loc = t0 - groups[g][2]
                xb = xbufs[g][:, loc : loc + nt, :]
                ob = obufs[i]
                g1b = g1_bf[:, None, 0:wd].broadcast_to([128, nt, wd])
                beb = be_bf[:, None, 0:wd].broadcast_to([128, nt, wd])
                nc.vector.tensor_tensor(ob, xb[:, :, 0:wd], g1b, mult)
                nc.vector.tensor_tensor(ob, ob, beb, add)
                engs[cfg["out_ring"][i]].dma_start(
                    out=out[:, t0 : t0 + nt, c0 : c0 + wd], in_=ob
                )
                if i + look < len(chunks):
                    emit_in_for(i + look)
    if finalize:
        nc.finalize()
    return nc


def make_core_inputs(x_cond, x_to_film, W, b, core):
    """Host-side shard + layout prep for one core (pure layout/dtype moves)."""
    sl = slice(core * BL, (core + 1) * BL)
    xct = x_cond[sl].T.reshape(KT, 128, BL).transpose(1, 0, 2)
    w_t = W.reshape(KT, 128, 2 * D_OUT).transpose(1, 0, 2)
    wx = np.concatenate([xct, w_t], axis=2)
    return {
        "wx": np.ascontiguousarray(wx).astype(np_bf16),
        "x_to_film": np.ascontiguousarray(x_to_film[sl]).astype(np_bf16),
        "b": np.concatenate(
            [b, np.ones(128, np.float32)]
        ).reshape(1, -1).astype(np_bf16),
    }


_NC_CACHE = []


def kernel(**inputs: np.ndarray) -> np.ndarray:
    x_cond = np.asarray(inputs["x_cond"], dtype=np.float32)
    x_to_film = np.asarray(inputs["x_to_film"], dtype=np.float32)
    W = np.asarray(inputs["W"], dtype=np.float32)
    b = np.asarray(inputs["b"], dtype=np.float32)

    if not _NC_CACHE:
        _NC_CACHE.append(build_core_module())
    nc = _NC_CACHE[0]

    in_maps = [
        make_core_inputs(x_cond, x_to_film, W, b, c) for c in range(N_CORES)
    ]
    res = run_bass_kernel_spmd(nc, in_maps, core_ids=list(range(N_CORES)))
    return np.concatenate(
        [np.asarray(r["out"]).astype(np.float32) for r in res.results], axis=0
    )


# revision 12
# speedup vs baseline: 1.0461x; 1.0461x over previous
"""Trainium2 Bass kernel: FiLM modulation + batched block-diagonal scatter.

Reference computation (per batch row):
    gb    = x_cond @ W + b                       # [172]
    gamma = gb[:86]; beta = gb[86:]
    out3d = (1 + gamma) * x_to_film + beta       # [256, 86]
    result[t, c] = block-diagonal placement: rows 0:86 -> cols 0:86,
                   rows 86:172 -> cols 86:172, rows 172:256 -> cols 172:256
                   (last block truncated to 84 cols); everything else zero.

Strategy: pure data parallel over the batch dim (1024 -> 8 cores x 128 rows).
Per core, batch rows live on the 128 SBUF partitions.

Performance structure (v2, bf16 datapath):
  - The film ops are DVE tensor_tensor; with every operand bf16 and packed
    along the innermost dim they run in the DVE 2x_1p perf mode (0.5
    cycles/elem instead of 1), halving the ~46us fp32 DVE floor to ~23us.
    x_to_film is loaded f32 from HBM and downcast to bf16 on the otherwise
    idle ACT engine (~19us, fully overlapped with DVE).
  - gb = x_cond @ W + b runs on PE in bf16 (1 cycle/row vs 4 for fp32).
    x_cond is pre-transposed on the host (pure layout change) so no PE
    transpose / PSUM round-trip sits on the critical path; gamma's PSUM
    accumulation group is separate from beta's so the first film multiply
    can start as soon as gamma lands.
  - Output blocks are written unpadded (86/84 cols = 172/168B descriptors).
    Sub-512B descriptors pay the documented 2x read-modify-write penalty,
    but at bf16 that equals the f32 padded-row cost with half the SBUF
    footprint and no margin-zeroing ops.
  - Three DMA queues (SP + ACT HWDGE, Pool SWDGE) transfer in parallel;
    same-queue transfers serialize, so loads/stores are spread across all
    three with the chunk splits and ring strings below (tuned via random
    search over the CoreSim cost model).
"""

import numpy as np

import concourse.bacc as bacc
import concourse.mybir as mybir
from concourse.bass_utils import run_bass_kernel_spmd

try:  # ml_dtypes provides the numpy bfloat16
    from ml_dtypes import bfloat16 as np_bf16
except ImportError:  # pragma: no cover
    import jax.numpy as jnp

    np_bf16 = jnp.bfloat16
from concourse.tile import TileContext

B, T, D_COND, D_OUT = 1024, 256, 768, 86
N_CORES = 8
BL = B // N_CORES  # 128 batch rows per core = SBUF partition count
KT = D_COND // 128  # 6 contraction tiles

# block structure of the output: (t_start, t_end, col_start, width)
BLOCKS = [(0, 86, 0, 86), (86, 172, 86, 86), (172, 256, 172, 84)]


def make_chunks(splits):
    """splits[b] = list of row counts for block b -> (t0, nt, c0, wd)."""
    chunks = []
    for (tb, te, c0, wd), ns in zip(BLOCKS, splits):
        assert sum(ns) == te - tb
        t = tb
        for n in ns:
            chunks.append((t, n, c0, wd))
            t += n
    return chunks


DEFAULT_CFG = {
    "splits": [[16, 35, 35], [43, 43], [42, 28, 14]],
    # per-chunk engine strings: S=sync(SP) A=scalar(ACT) P=gpsimd(Pool)
    "in_ring": "APPPPPPP",
    "out_ring": "ASASASAS",
    "in_group": None,
    "wx_ring": "SA",
    "b_ring": "S",
    "lookahead": 3,
}


def build_core_module(finalize=True, cfg=DEFAULT_CFG):
    nc = bacc.Bacc(
        "TRN2", target_bir_lowering=False, debug=False, enable_asserts=False
    )
    f32 = mybir.dt.float32
    bf16 = mybir.dt.bfloat16
    mult = mybir.AluOpType.mult
    add = mybir.AluOpType.add
    chunks = make_chunks(cfg["splits"])
    # Packed gb operands, one load: wx[:, k, 0:128] = x_cond^T k-tile
    # (xct[b_part, k, b] = x_cond[b, k*128 + b_part]), wx[:, k, 128:300] =
    # W k-tile (host layout prep, pure transpose/pack).
    wx = nc.dram_tensor(
        "wx", [128, KT, 128 + 2 * D_OUT], bf16, kind="ExternalInput"
    )
    xf = nc.dram_tensor("x_to_film", [BL, T, D_OUT], bf16, kind="ExternalInput")
    # b packed with a row of ones (cols 172:300) used as the K=1 lhsT
    # for the bias rank-1 matmul - saves the on-device memset.
    bv = nc.dram_tensor("b", [1, 2 * D_OUT + 128], bf16, kind="ExternalInput")
    out = nc.dram_tensor("out", [BL, T, T], bf16, kind="ExternalOutput")

    engs = {"S": nc.sync, "A": nc.scalar, "P": nc.gpsimd, "V": nc.vector}

    with TileContext(nc) as tc:
        with (
            tc.tile_pool(name="persist", bufs=1) as persist,
            tc.tile_pool(name="gbps", bufs=1, space="PSUM") as gbps,
            tc.tile_pool(name="work", bufs=3) as work,
        ):
            # --- gb = x_cond @ W + b (PE, bf16 operands, f32 PSUM accum) ---
            # gamma and beta accumulate in separate PSUM groups so gamma (the
            # first film operand needed) posts without waiting for beta.
            g1_bf = persist.tile([128, D_OUT], bf16, tag="g1")
            be_bf = persist.tile([128, D_OUT], bf16, tag="be")
            with tc.tile_pool(name="setup", bufs=1) as setup:
                wx_sb = setup.tile([128, KT, 128 + 2 * D_OUT], bf16)
                wxr = cfg["wx_ring"]
                if len(wxr) == 1:
                    engs[wxr].dma_start(out=wx_sb, in_=wx[:, :, :])
                else:
                    h = KT // 2
                    engs[wxr[0]].dma_start(
                        out=wx_sb[:, 0:h, :], in_=wx[:, 0:h, :]
                    )
                    engs[wxr[1]].dma_start(
                        out=wx_sb[:, h:KT, :], in_=wx[:, h:KT, :]
                    )
                xct_sb = wx_sb[:, :, 0:128]
                w_sb = wx_sb[:, :, 128:]
                b_sb = setup.tile([1, 2 * D_OUT + 128], bf16)
                engs[cfg["b_ring"]].dma_start(out=b_sb, in_=bv[:, :])
                ones = b_sb[:, 2 * D_OUT :]

                g_ps = gbps.tile([128, D_OUT], f32, tag="g_ps")
                b_ps = gbps.tile([128, D_OUT], f32, tag="b_ps")
                for k in range(KT):
                    nc.tensor.matmul(
                        g_ps,
                        xct_sb[:, k, :],
                        w_sb[:, k, 0:D_OUT],
                        start=(k == 0),
                        stop=False,
                    )
                nc.tensor.matmul(
                    g_ps, ones, b_sb[:, 0:D_OUT], start=False, stop=True
                )
                # gb[:, :86] -> 1+gamma (bf16), for the film multiply.
                # On DVE (idle during fill) this avoids a cross-engine sem
                # hop and can't be blocked behind an ACT-ring DMA transfer.
                if cfg.get("g1_eng", "V") == "V":
                    nc.vector.tensor_scalar(g1_bf, g_ps, 1.0, None, add)
                else:
                    nc.scalar.add(g1_bf, g_ps, 1.0)
                for k in range(KT):
                    nc.tensor.matmul(
                        b_ps,
                        xct_sb[:, k, :],
                        w_sb[:, k, D_OUT:],
                        start=(k == 0),
                        stop=False,
                    )
                nc.tensor.matmul(
                    b_ps, ones, b_sb[:, D_OUT : 2 * D_OUT], start=False, stop=True
                )
                if cfg.get("be_eng", "A") == "V":
                    nc.vector.tensor_scalar(be_bf, b_ps, 0.0, None, add)
                else:
                    nc.scalar.copy(be_bf, b_ps)

            # --- FiLM + block writes ---
            # Per chunk: f32 load -> ACT downcast to bf16 -> two DVE
            # tensor_tensor passes in 2x mode -> unpadded block write.
            obufs = []
            for i, (t0, nt, c0, wd) in enumerate(chunks):
                ob = persist.tile([128, nt, wd], bf16, tag=f"obuf{i}")
                obufs.append(ob)
            # Input loads are decoupled from film chunks: x_to_film is
            # contiguous in t, so one load can span several film chunks
            # (and block boundaries), amortizing the ~1us per-DMA ring
            # overhead. cfg["in_group"][g] = number of consecutive film
            # chunks covered by load g.
            in_group = cfg.get("in_group") or [1] * len(chunks)
            assert sum(in_group) == len(chunks)
            groups = []  # (first_chunk, n_chunks, t0, nt_total)
            ci = 0
            for g, cnt in enumerate(in_group):
                t0 = chunks[ci][0]
                ntt = sum(c[1] for c in chunks[ci : ci + cnt])
                groups.append((ci, cnt, t0, ntt))
                ci += cnt
            group_of = {}
            for g, (c0i, cnt, t0, ntt) in enumerate(groups):
                for i in range(c0i, c0i + cnt):
                    group_of[i] = g
            xbufs = [
                persist.tile(
                    [128, ntt, D_OUT], bf16, tag=f"xb{g}", name=f"xb{g}"
                )
                for g, (c0i, cnt, t0, ntt) in enumerate(groups)
            ]
            # Emission order sets per-ring DMA queue order; a queued DMA
            # whose data isn't ready blocks its ring, so loads are emitted
            # `lookahead` film-chunks ahead of the film ops consuming them.
            look = cfg.get("lookahead", 3)
            emitted = set()

            def emit_in_for(i):
                g = group_of[min(i, len(chunks) - 1)]
                if g in emitted:
                    return
                emitted.add(g)
                _, _, gt0, gnt = groups[g]
                engs[cfg["in_ring"][g]].dma_start(
                    out=xbufs[g], in_=xf[:, gt0 : gt0 + gnt, :]
                )

            for j in range(min(look, len(chunks))):
                emit_in_for(j)
            for i, (t0, nt, c0, wd) in enumerate(chunks):
                g = group_of[i]
                loc = t0 - groups[g][2]
                xb = xbufs[g][:, loc : loc + nt, :]
                ob = obufs[i]
                g1b = g1_bf[:, None, 0:wd].broadcast_to([128, nt, wd])
                beb = be_bf[:, None, 0:wd].broadcast_to([128, nt, wd])
                nc.vector.tensor_tensor(ob, xb[:, :, 0:wd], g1b, mult)
                nc.vector.tensor_tensor(ob, ob, beb, add)
                engs[cfg["out_ring"][i]].dma_start(
                    out=out[:, t0 : t0 + nt, c0 : c0 + wd], in_=ob
                )
                if i + look < len(chunks):
                    emit_in_for(i + look)
    if finalize:
        nc.finalize()
    return nc


def make_core_inputs(x_cond, x_to_film, W, b, core):
    """Host-side shard + layout prep for one core (pure layout/dtype moves)."""
    sl = slice(core * BL, (core + 1) * BL)
    xct = x_cond[sl].T.reshape(KT, 128, BL).transpose(1, 0, 2)
    w_t = W.reshape(KT, 128, 2 * D_OUT).transpose(1, 0, 2)
    wx = np.concatenate([xct, w_t], axis=2)
    return {
        "wx": np.ascontiguousarray(wx).astype(np_bf16),
        "x_to_film": np.ascontiguousarray(x_to_film[sl]).astype(np_bf16),
        "b": np.concatenate(
            [b, np.ones(128, np.float32)]
        ).reshape(1, -1).astype(np_bf16),
    }


_NC_CACHE = []


def kernel(**inputs: np.ndarray) -> np.ndarray:
    x_cond = np.asarray(inputs["x_cond"], dtype=np.float32)
    x_to_film = np.asarray(inputs["x_to_film"], dtype=np.float32)
    W = np.asarray(inputs["W"], dtype=np.float32)
    b = np.asarray(inputs["b"], dtype=np.float32)

    if not _NC_CACHE:
        _NC_CACHE.append(build_core_module())
    nc = _NC_CACHE[0]

    in_maps = [
        make_core_inputs(x_cond, x_to_film, W, b, c) for c in range(N_CORES)
    ]
    res = run_bass_kernel_spmd(nc, in_maps, core_ids=list(range(N_CORES)))
    return np.concatenate(
        [np.asarray(r["out"]).astype(np.float32) for r in res.results], axis=0
    )
